# revision 5
# baseline (speedup 1.0000x reference)
"""Llama attention (B=2, S=2048, H=2048, NH=16, HD=128) on 8 Trainium2 cores.

Sharding: tensor-parallel over heads x data-parallel over batch.
  core c -> batch b = c//4, head group g = c%4 (heads 4g..4g+3).

Each core computes, for its (batch, 4 heads):
  - qT/kT/vnat projections (fp32r matmuls, contraction over H in 128-chunks)
  - RoPE on q, k via cross-partition-base DVE ops (host-precomputed cos/sin)
  - flash-style attention with scores transposed [sk, sq]:
      scoresT = kT.T-slices @ qT-chunk, exp on ACT (bf16 out), PV and
      ones-rowsum accumulate in PSUM, normalize via K=1 broadcast matmul
  - partial o_proj over its head slice -> out_p [S, H]
Host sums out_p over the 4 cores of each batch and assembles k, v outputs.

Mask handling: score tiles [sk=128, sq=512] are classified host-side from the
actual attention_mask into plain / fully-masked (skipped) / mixed (additive
mask tile applied). The program is specialized to the classification and
cached; a causal mask yields 4 unique mixed tiles and ~47% tile skips.
"""
import sys

try:
    import concourse  # noqa: F401
except ImportError:
    sys.path.insert(0, "/opt/trn_rl_repo")

import numpy as np

import concourse.tile as tile
from concourse import bacc, mybir
from concourse.bass_utils import run_bass_kernel_spmd

F32 = mybir.dt.float32
F32R = mybir.dt.float32r
BF16 = mybir.dt.bfloat16
EXP = mybir.ActivationFunctionType.Exp
ADD = mybir.AluOpType.add
MULT = mybir.AluOpType.mult

B, S, H = 2, 2048, 2048
NH, HD = 16, 128
ROPE_THETA = 10000.0
N_CORES = 8
HEADS_PER_CORE = NH // (N_CORES // B)  # 4
ESL = HEADS_PER_CORE * HD              # 512, per-core head-feature slice
NCH = S // 512                         # 4 s-chunks of 512
NKT = S // 128                         # 16 sk tiles
NHC = H // 128                         # 16 contraction chunks
SCALE = 1.0 / np.sqrt(HD)
SQ_LOOKAHEAD = 2

# score-tile classes
PLAIN, SKIP, MIXED = 0, 1, 2


def _classify_mask(mask2d):
    """Classify [sk-tile i, sq-chunk j] blocks of the additive mask.

    Returns (classes[i][j], unique_tiles [n,128,512] scaled by 1/SCALE,
    tile_idx[i][j]). mask2d is mask[q, k]; our tiles are transposed [sk, sq].
    """
    classes = [[PLAIN] * NCH for _ in range(NKT)]
    uniq = {}
    tile_idx = [[-1] * NCH for _ in range(NKT)]
    tiles = []
    for i in range(NKT):
        for j in range(NCH):
            blk = mask2d[j * 512:(j + 1) * 512, i * 128:(i + 1) * 128]
            if not blk.any():
                classes[i][j] = PLAIN
            elif (blk <= -1e8).all():
                classes[i][j] = SKIP
            else:
                classes[i][j] = MIXED
                t = np.ascontiguousarray(blk.T) * np.float32(1.0 / SCALE)
                key = t.tobytes()
                if key not in uniq:
                    uniq[key] = len(tiles)
                    tiles.append(t)
                tile_idx[i][j] = uniq[key]
    for j in range(NCH):
        assert any(classes[i][j] != SKIP for i in range(NKT)), (
            "fully-masked query chunk: softmax undefined without max-subtract"
        )
    ut = (np.stack(tiles) if tiles
          else np.zeros((0, 128, 512), np.float32)).astype(np.float32)
    return classes, ut, tile_idx


def _rope_tables(position_ids_b):
    """cosT [128, S] and sign-baked sinT [128, S] fp32 for one batch row."""
    inv_freq = 1.0 / (ROPE_THETA ** (np.arange(0, HD, 2, dtype=np.float32) / HD))
    freqs = position_ids_b.astype(np.float32)[:, None] * inv_freq[None, :]  # [S,64]
    emb = np.concatenate([freqs, freqs], axis=1)  # [S,128]
    cosT = np.ascontiguousarray(np.cos(emb).T).astype(np.float32)
    sinT = np.sin(emb).T
    sinT = np.concatenate([-sinT[:64], sinT[64:]], axis=0)
    return cosT, np.ascontiguousarray(sinT).astype(np.float32)


def _build_program(classes, tile_idx, n_mask_tiles):
    """Build the single-core Bass program (same for all cores; data differs)."""
    nc = bacc.Bacc("TRN2", target_bir_lowering=False, debug=False)

    hid_d = nc.dram_tensor("hidT", [H, S], F32R, kind="ExternalInput")
    wk_d = nc.dram_tensor("wkT", [H, ESL], F32R, kind="ExternalInput")
    wv_d = nc.dram_tensor("wvT", [H, ESL], F32R, kind="ExternalInput")
    wq_d = nc.dram_tensor("wqT", [H, ESL], F32R, kind="ExternalInput")
    wo_d = nc.dram_tensor("woT", [ESL, H], F32R, kind="ExternalInput")
    cos_d = nc.dram_tensor("cosT", [HD, S], F32, kind="ExternalInput")
    sin_d = nc.dram_tensor("sinT", [HD, S], F32, kind="ExternalInput")
    if n_mask_tiles:
        mask_d = nc.dram_tensor(
            "maskTiles", [n_mask_tiles, 128, 512], F32, kind="ExternalInput"
        )

    onesc_d = nc.dram_tensor("onesCol", [128, 1], BF16, kind="ExternalInput")
    onesr_d = nc.dram_tensor("onesRow", [1, 128], F32R, kind="ExternalInput")

    out_d = nc.dram_tensor("out_p", [S, H], F32, kind="ExternalOutput")
    k_d = nc.dram_tensor("k_out", [ESL, S], F32R, kind="ExternalOutput")
    v_d = nc.dram_tensor("v_out", [S, ESL], F32, kind="ExternalOutput")

    with tile.TileContext(nc) as tc:
        with (
            nc.allow_low_precision(reason="fp32r feeds full-rate PE matmuls"),
            tc.tile_pool(name="sb", bufs=1) as sb,
            tc.tile_pool(name="psp", bufs=8, space="PSUM") as psp,
        ):
            def sbt(tag, bufs, dtype=F32, shape=(128, 512)):
                return sb.tile(list(shape), dtype, tag=tag, bufs=bufs,
                               name=tag)

            def pst(shape=(128, 512)):
                return psp.tile(list(shape), F32, tag="ps", bufs=8, name="ps")

            # constants (DMA'd: memset cannot write f32r)
            ones_bf = sbt("ones_bf", 1, BF16, (128, 1))
            nc.sync.dma_start(ones_bf[:], onesc_d.ap())
            ones_row = sbt("ones_row", 1, F32R, (1, 128))
            nc.sync.dma_start(ones_row[:], onesr_d.ap())

            # resident output-side tiles
            kT = {}   # (h, j) -> [128, 512] f32 (d, s-chunk)
            qT = {}
            vbf = {}  # s-tile t -> [128, 512] bf16 (s, head-feats)

            def load_w(dram, n, tag="w"):
                ts = []
                for c in range(n):
                    t = sbt(tag, 20, F32R)
                    nc.sync.dma_start(t[:], dram.ap()[c * 128:(c + 1) * 128, :])
                    ts.append(t)
                return ts

            def rope(ps_in, dst, cos_t, sin_t, tmp):
                # dst = ps_in * cos + rotate_half(ps_in) * sin_signed
                nc.vector.tensor_tensor(
                    tmp[0:64, :], ps_in[64:128, :], sin_t[0:64, :], MULT)
                nc.vector.tensor_tensor(
                    tmp[64:128, :], ps_in[0:64, :], sin_t[64:128, :], MULT)
                nc.vector.tensor_tensor(dst[:], ps_in[:], cos_t[:], MULT)
                nc.vector.tensor_tensor(dst[:], dst[:], tmp[:], ADD)

            # ---- Phase A: projections ----
            with nc.named_scope("proj_k"):
                wts = load_w(wk_d, NHC)
                for j in range(NCH):
                    cos_t = sbt("cs", 4)
                    nc.sync.dma_start(
                        cos_t[:], cos_d.ap()[:, j * 512:(j + 1) * 512])
                    sin_t = sbt("cs", 4)
                    nc.sync.dma_start(
                        sin_t[:], sin_d.ap()[:, j * 512:(j + 1) * 512])
                    kps = [pst() for _ in range(4)]
                    for hc in range(NHC):
                        hid_t = sbt("hid", 6, F32R)
                        nc.sync.dma_start(
                            hid_t[:],
                            hid_d.ap()[hc * 128:(hc + 1) * 128,
                                       j * 512:(j + 1) * 512])
                        for g in range(4):
                            nc.tensor.matmul(
                                kps[g][:],
                                wts[hc][:, g * 128:(g + 1) * 128],
                                hid_t[:],
                                start=(hc == 0), stop=(hc == NHC - 1))
                    for g in range(4):
                        dst = sbt("kT", 16, F32R)
                        tmp = sbt("rtmp", 3)
                        rope(kps[g], dst, cos_t, sin_t, tmp)
                        kT[(g, j)] = dst
                        nc.sync.dma_start(
                            k_d.ap()[g * 128:(g + 1) * 128,
                                     j * 512:(j + 1) * 512], dst[:])

            with nc.named_scope("proj_v"):
                wts = load_w(wv_d, NHC)
                for j in range(NCH):
                    vps = [pst() for _ in range(4)]
                    for hc in range(NHC):
                        hid_t = sbt("hid", 6, F32R)
                        nc.sync.dma_start(
                            hid_t[:],
                            hid_d.ap()[hc * 128:(hc + 1) * 128,
                                       j * 512:(j + 1) * 512])
                        for t in range(4):
                            nc.tensor.matmul(
                                vps[t][:],
                                hid_t[:, t * 128:(t + 1) * 128],
                                wts[hc][:],
                                start=(hc == 0), stop=(hc == NHC - 1))
                    for t in range(4):
                        st = 4 * j + t
                        vb = sbt("vbf", 16, BF16)
                        nc.vector.tensor_copy(vb[:], vps[t][:])
                        vbf[st] = vb
                        stg = sbt("stage", 6)
                        nc.scalar.copy(stg[:], vps[t][:])
                        nc.sync.dma_start(
                            v_d.ap()[st * 128:(st + 1) * 128, :], stg[:])

            with nc.named_scope("proj_q"):
                wts = load_w(wq_d, NHC)
                for j in range(NCH):
                    cos_t = sbt("cs", 4)
                    nc.sync.dma_start(
                        cos_t[:], cos_d.ap()[:, j * 512:(j + 1) * 512])
                    sin_t = sbt("cs", 4)
                    nc.sync.dma_start(
                        sin_t[:], sin_d.ap()[:, j * 512:(j + 1) * 512])
                    qps = [pst() for _ in range(4)]
                    for hc in range(NHC):
                        hid_t = sbt("hid", 6, F32R)
                        nc.sync.dma_start(
                            hid_t[:],
                            hid_d.ap()[hc * 128:(hc + 1) * 128,
                                       j * 512:(j + 1) * 512])
                        for g in range(4):
                            nc.tensor.matmul(
                                qps[g][:],
                                wts[hc][:, g * 128:(g + 1) * 128],
                                hid_t[:],
                                start=(hc == 0), stop=(hc == NHC - 1))
                    for g in range(4):
                        dst = sbt("qT", 16, F32R)
                        tmp = sbt("rtmp", 3)
                        rope(qps[g], dst, cos_t, sin_t, tmp)
                        qT[(g, j)] = dst

            # ---- Phase B: attention + o_proj ----
            wo_t = {}  # (h, ec)
            for h in range(4):
                for ec in range(4):
                    t = sbt("w", 20, F32R)
                    nc.sync.dma_start(
                        t[:], wo_d.ap()[h * 128:(h + 1) * 128,
                                        ec * 512:(ec + 1) * 512])
                    wo_t[(h, ec)] = t

            mask_t = {}
            if n_mask_tiles:
                for m in range(n_mask_tiles):
                    t = sbt("mask", max(n_mask_tiles, 1))
                    nc.sync.dma_start(t[:], mask_d.ap()[m])
                    mask_t[m] = t

            with nc.named_scope("attn"):
                for j in range(NCH):
                    at_tiles = {}
                    for h in range(4):
                        active = [i for i in range(NKT)
                                  if classes[i][j] != SKIP]
                        n_act = len(active)
                        pv = pst()
                        rs = pst((1, 512))
                        pt_tiles = {}

                        def emit_score(idx):
                            i = active[idx]
                            sc = pst()
                            nc.tensor.matmul(
                                sc[:],
                                kT[(h, i // 4)][:, (i % 4) * 128:
                                                (i % 4 + 1) * 128],
                                qT[(h, j)][:],
                                start=True, stop=True)
                            if classes[i][j] == MIXED:
                                nc.vector.tensor_tensor(
                                    sc[:], sc[:], mask_t[tile_idx[i][j]][:],
                                    ADD)
                            pt = sbt("pt", 16, BF16)
                            nc.scalar.activation(
                                pt[:], sc[:], EXP, scale=float(SCALE))
                            pt_tiles[idx] = pt

                        for idx in range(min(SQ_LOOKAHEAD, n_act)):
                            emit_score(idx)
                        for idx in range(n_act):
                            if idx + SQ_LOOKAHEAD < n_act:
                                emit_score(idx + SQ_LOOKAHEAD)
                            i = active[idx]
                            pt = pt_tiles.pop(idx)
                            st = i  # sk tile index == s tile index of v
                            nc.tensor.matmul(
                                pv[:],
                                vbf[st][:, h * 128:(h + 1) * 128],
                                pt[:],
                                start=(idx == 0), stop=(idx == n_act - 1))
                            nc.tensor.matmul(
                                rs[:], ones_bf[:], pt[:],
                                start=(idx == 0), stop=(idx == n_act - 1))

                        recip = sbt("recip", 2, F32R, (1, 512))
                        nc.vector.reciprocal(recip[:], rs[:])
                        bc = pst()
                        nc.tensor.matmul(
                            bc[:], ones_row[:],
                            recip[:], start=True, stop=True)
                        bcs = sbt("rtmp", 3)
                        nc.scalar.copy(bcs[:], bc[:])
                        att = sbt("at", 8, F32R)
                        nc.vector.tensor_tensor(att[:], pv[:], bcs[:], MULT)
                        at_tiles[h] = att

                    with nc.named_scope("oproj"):
                        for t in range(4):
                            for ec in range(4):
                                op = pst()
                                for h in range(4):
                                    nc.tensor.matmul(
                                        op[:],
                                        at_tiles[h][:, t * 128:
                                                    (t + 1) * 128],
                                        wo_t[(h, ec)][:],
                                        start=(h == 0), stop=(h == 3))
                                stg = sbt("stage", 6)
                                nc.scalar.copy(stg[:], op[:])
                                nc.sync.dma_start(
                                    out_d.ap()[j * 512 + t * 128:
                                               j * 512 + (t + 1) * 128,
                                               ec * 512:(ec + 1) * 512],
                                    stg[:])

    nc.compile()
    return nc


_PROGRAM_CACHE = {}

import ml_dtypes as _mld
_ONES_COL = np.ones((128, 1), dtype=_mld.bfloat16)
_ONES_ROW = np.ones((1, 128), dtype=np.float32)


def kernel(hidden_states, attention_mask, position_ids, Wq, Wk, Wv, Wo):
    hidden_states = np.asarray(hidden_states, dtype=np.float32)
    attention_mask = np.asarray(attention_mask, dtype=np.float32)
    position_ids = np.asarray(position_ids)
    Wq = np.asarray(Wq, dtype=np.float32)
    Wk = np.asarray(Wk, dtype=np.float32)
    Wv = np.asarray(Wv, dtype=np.float32)
    Wo = np.asarray(Wo, dtype=np.float32)

    classes, mask_tiles, tile_idx = _classify_mask(attention_mask[0, 0])
    n_mask = mask_tiles.shape[0]

    cache_key = (
        tuple(tuple(r) for r in classes),
        tuple(tuple(r) for r in tile_idx),
    )
    if cache_key not in _PROGRAM_CACHE:
        _PROGRAM_CACHE[cache_key] = _build_program(classes, tile_idx, n_mask)
    nc = _PROGRAM_CACHE[cache_key]

    rope_tabs = [_rope_tables(position_ids[b]) for b in range(B)]

    in_maps = []
    for c in range(N_CORES):
        b, g = c // 4, c % 4
        sl = slice(g * ESL, (g + 1) * ESL)
        m = {
            "onesCol": _ONES_COL,
            "onesRow": _ONES_ROW,
            "hidT": np.ascontiguousarray(hidden_states[b].T),
            "wkT": np.ascontiguousarray(Wk[sl, :].T),
            "wvT": np.ascontiguousarray(Wv[sl, :].T),
            "wqT": np.ascontiguousarray(Wq[sl, :].T),
            "woT": np.ascontiguousarray(Wo[:, sl].T),
            "cosT": rope_tabs[b][0],
            "sinT": rope_tabs[b][1],
        }
        if n_mask:
            m["maskTiles"] = mask_tiles
        in_maps.append(m)

    res = run_bass_kernel_spmd(nc, in_maps, core_ids=list(range(N_CORES)))

    out = np.zeros((B, S, H), np.float32)
    k = np.empty((B, NH, S, HD), np.float32)
    v = np.empty((B, NH, S, HD), np.float32)
    for c in range(N_CORES):
        b, g = c // 4, c % 4
        r = res.results[c]
        out[b] += r["out_p"]
        ko = r["k_out"]  # [ESL, S] rows = (h, d)
        vo = r["v_out"]  # [S, ESL] cols = (h, d)
        for h in range(HEADS_PER_CORE):
            k[b, 4 * g + h] = ko[h * 128:(h + 1) * 128, :].T
            v[b, 4 * g + h] = vo[:, h * 128:(h + 1) * 128]
    return out, k, v


if __name__ == "__main__":
    rng = np.random.default_rng(0)
    hs = rng.standard_normal((B, S, H), dtype=np.float32)
    causal = np.tril(np.ones((S, S), bool))
    am = np.where(causal, 0.0, -1e9).astype(np.float32)[None, None]
    pid = np.broadcast_to(np.arange(S, dtype=np.int32)[None], (B, S)).copy()
    sc = 1.0 / np.sqrt(H)
    ws = [rng.standard_normal((H, H), dtype=np.float32) * sc for _ in range(4)]
    o, kk, vv = kernel(hs, am, pid, *ws)
    print("out", o.shape, "k", kk.shape, "v", vv.shape)
